# revision 1
# baseline (speedup 1.0000x reference)
"""Causal self-attention (B=4, S=2048, D=1024, H=16) on 8 trn2 cores.

Sharding: core c = 2*b + g  (b = batch 0..3, g = head-group 0..1, 8 heads/group).
Each core computes, for its batch element and its 8 heads:
    qkv -> causal attention -> y @ w_proj[rows of its head group]
The two head-group partial outputs per batch are summed on the host.

Device layouts (all fp32; matmuls run as float32r via bitcast):
    xT [D, S]       x[b] transposed on host (contraction dim on partitions)
    wq/wk/wv [D, 512]  w_qkv column slices for the group
    wp [512, D]     w_proj row slice
On-chip: Q^T,K^T in [head_dim, s] layout; scores computed transposed
([sk, sq]) so exp(p) feeds the AV matmul directly as the moving operand;
an all-ones column appended to V gives the softmax denominator for free
(row 64 of the AV psum).  Normalization is deferred to Y via a DRAM
round-trip broadcast of 1/l.
"""

import numpy as np

import concourse.mybir as mybir
import concourse.tile as tile
from concourse import bacc
from concourse.bass_utils import run_bass_kernel_spmd

P = 128
D = 1024
KD = D // P          # 8 contraction chunks
GCOLS = 512          # qkv cols per head group
HG = 8               # heads per core
HD = 64
NJ = 4               # head-pair col tiles (2 heads x 64 = 128)
SQT = 512            # sq tile (matmul moving dim)
F32 = mybir.dt.float32
F32R = mybir.dt.float32r

TRACE = False
PPOOL_BUFS = 3
SC_BUFS = 3
ST_BUFS = 3
QKT_BUFS = 2
YP_BUFS = 2
MASK_ON_DVE = True
WS_BUFS = 2
PJ_BUFS = 5
TRACE_KWARGS = {}


def _r(ap):
    return ap.bitcast(F32R)


def build_nc(S=2048):
    NT4 = S // SQT       # sq tiles of 512
    NT16 = S // P        # s chunks of 128
    nc = bacc.Bacc("TRN2", target_bir_lowering=False, debug=False)

    xT = nc.dram_tensor("xT", [D, S], F32R, kind="ExternalInput").ap()
    wq = nc.dram_tensor("wq", [D, GCOLS], F32R, kind="ExternalInput").ap()
    wk = nc.dram_tensor("wk", [D, GCOLS], F32R, kind="ExternalInput").ap()
    wv = nc.dram_tensor("wv", [D, GCOLS], F32R, kind="ExternalInput").ap()
    wp = nc.dram_tensor("wp", [GCOLS, D], F32R, kind="ExternalInput").ap()
    mk = nc.dram_tensor("mk", [P, 4, SQT], F32R, kind="ExternalInput").ap()
    out = nc.dram_tensor("out", [S, D], F32, kind="ExternalOutput").ap()

    with tile.TileContext(nc) as tc:
        with (
            tc.tile_pool(name="persist", bufs=1) as persist,
            tc.tile_pool(name="qkt", bufs=QKT_BUFS) as qkt,
            tc.tile_pool(name="ppool", bufs=PPOOL_BUFS) as ppool,
            tc.tile_pool(name="stpool", bufs=ST_BUFS) as stpool,
            tc.tile_pool(name="btpool", bufs=ST_BUFS) as btpool,
            tc.tile_pool(name="dram", bufs=1, space="DRAM") as drampool,
            tc.tile_pool(name="ps_sc", bufs=SC_BUFS, space="PSUM") as ps_sc,
            tc.tile_pool(name="ps_y", bufs=YP_BUFS, space="PSUM") as ps_y,
        ):
            V = persist.tile([P, NT16, HG, HD + 1], F32R)
            MK = persist.tile([P, 4, SQT], F32R)
            nc.sync.dma_start(out=MK, in_=mk)
            ld = drampool.tile([HG, S], F32R)
            yd = drampool.tile([GCOLS, S], F32R)

            qT_sb = {}
            kT_sb = {}
            wq_sb = {}
            wk_sb = {}

            def qk_tile(j, t):
                """Q^T,K^T matmuls for head-pair col-tile j, s-tile t."""
                xTs = qk_tile.xTs
                if t == 0:
                    wqj = wstream.tile([P, KD, P], F32R, tag="wqj")
                    wkj = wstream.tile([P, KD, P], F32R, tag="wkj")
                    nc.sync.dma_start(
                        out=wqj,
                        in_=wq[:, j * P : (j + 1) * P].rearrange(
                            "(k p) c -> p k c", p=P
                        ),
                    )
                    nc.sync.dma_start(
                        out=wkj,
                        in_=wk[:, j * P : (j + 1) * P].rearrange(
                            "(k p) c -> p k c", p=P
                        ),
                    )
                    wq_sb[j] = wqj
                    wk_sb[j] = wkj
                    qT_sb[j] = qkt.tile([P, S], F32R, name="qTj", tag="qTj")
                    kT_sb[j] = qkt.tile([P, S], F32R, name="kTj", tag="kTj")
                for wsb, dest in (
                    (wq_sb[j], qT_sb[j]),
                    (wk_sb[j], kT_sb[j]),
                ):
                    ps = ps_sc.tile([P, SQT], F32, name="ps_qk", tag="sc")
                    for k in range(KD):
                        nc.tensor.matmul(
                            ps,
                            lhsT=_r(wsb[:, k, :]),
                            rhs=_r(xTs[:, k, t * SQT : (t + 1) * SQT]),
                            start=(k == 0),
                            stop=(k == KD - 1),
                        )
                    nc.vector.tensor_copy(
                        out=dest[:, t * SQT : (t + 1) * SQT], in_=ps
                    )

            def attn_tile(j, t, fillers=()):
                """Scores+softmax+AV for heads (2j, 2j+1) on sq tile t.

                Software-pipelined: AV of group g is emitted after the
                scores+exp of group g+1 so PE has work while ACT runs."""
                qTj = qT_sb[j]
                kTj = kT_sb[j]
                nch = 4 * t + 4  # causal sk chunks of 128
                yps = {}
                for hi in (0, 1):
                    h = 2 * j + hi
                    yps[h] = ps_y.tile(
                        [HD + 1, SQT], F32, name="yps", tag="yps"
                    )

                def chunk_off(c):
                    # causal column offset within the sq tile (trimmed; kept
                    # >=256 wide so f32r matmuls stay at full rate)
                    if c < 4 * t:
                        return 0
                    return (0, P, 2 * P, 2 * P)[c - 4 * t]

                def emit_scores_exp(g):
                    w = min(2, nch - g)
                    offs = [chunk_off(g + ci) for ci in range(w)]
                    pts = {}
                    for hi in (0, 1):
                        h = 2 * j + hi
                        base = HD * hi
                        sc = ps_sc.tile([P, 2 * SQT], F32, name="sc", tag="sc")
                        for ci in range(w):
                            c = g + ci
                            off = offs[ci]
                            nc.tensor.matmul(
                                sc[:, ci * SQT + off : (ci + 1) * SQT],
                                lhsT=_r(kTj[base : base + HD, c * P : (c + 1) * P]),
                                rhs=_r(
                                    qTj[
                                        base : base + HD,
                                        t * SQT + off : (t + 1) * SQT,
                                    ]
                                ),
                                start=True,
                                stop=True,
                            )
                        p = ppool.tile([P, 2 * SQT], F32R, name="pexp")
                        if all(o == 0 for o in offs):
                            nc.scalar.activation(
                                out=p[:, : w * SQT],
                                in_=sc[:, : w * SQT],
                                func=mybir.ActivationFunctionType.Exp,
                                scale=0.125,
                            )
                        else:
                            for ci in range(w):
                                off = offs[ci]
                                nc.scalar.activation(
                                    out=p[:, ci * SQT + off : (ci + 1) * SQT],
                                    in_=sc[:, ci * SQT + off : (ci + 1) * SQT],
                                    func=mybir.ActivationFunctionType.Exp,
                                    scale=0.125,
                                )
                        for ci in range(w):
                            c = g + ci
                            if c >= 4 * t:  # diagonal: zero non-causal
                                m = c - 4 * t
                                off = offs[ci]
                                wd = SQT - off
                                psl = p[:, ci * SQT + off : (ci + 1) * SQT]
                                if MASK_ON_DVE and hi == 0:
                                    nc.vector.tensor_mul(
                                        psl, psl, MK[:, m, off:SQT]
                                    )
                                else:
                                    nc.gpsimd.affine_select(
                                        out=psl,
                                        in_=psl,
                                        compare_op=mybir.AluOpType.is_ge,
                                        fill=0.0,
                                        base=off - P * m,
                                        channel_multiplier=-1,
                                        pattern=[[1, wd]],
                                    )
                        pts[h] = p
                    return pts

                def emit_av(g, pts):
                    w = min(2, nch - g)
                    for hi in (0, 1):
                        h = 2 * j + hi
                        for ci in range(w):
                            c = g + ci
                            off = chunk_off(c)
                            nc.tensor.matmul(
                                yps[h][:, off:SQT],
                                lhsT=_r(V[:, c, h, :]),
                                rhs=_r(pts[h][:, ci * SQT + off : (ci + 1) * SQT]),
                                start=(c == 0),
                                stop=(c == nch - 1),
                            )

                fill_iter = iter(fillers)
                prev = None
                for g in range(0, nch, 2):
                    pts = emit_scores_exp(g)
                    if prev is not None:
                        emit_av(*prev)
                    f = next(fill_iter, None)
                    if f is not None:
                        f()
                    prev = (g, pts)
                emit_av(*prev)
                for f in fill_iter:
                    f()

                for hi in (0, 1):
                    h = 2 * j + hi
                    st = stpool.tile([HD + 1, SQT], F32R, name="st")
                    nc.vector.tensor_copy(out=st, in_=yps[h])
                    nc.sync.dma_start(
                        out=ld[h : h + 1, t * SQT : (t + 1) * SQT],
                        in_=st[HD : HD + 1, :],
                    )
                    bt = btpool.tile([HD, SQT], F32R, name="bt")
                    nc.sync.dma_start(
                        out=bt,
                        in_=ld[
                            h : h + 1, t * SQT : (t + 1) * SQT
                        ].to_broadcast([HD, SQT]),
                    )
                    with nc.allow_low_precision(reason="tf32 1/l"):
                        nc.vector.reciprocal(out=bt, in_=bt)
                    nc.vector.tensor_mul(st[0:HD, :], st[0:HD, :], bt)
                    if j == NJ - 1:
                        ydst = attn_tile.y3[
                            HD * hi : HD * (hi + 1), t * SQT : (t + 1) * SQT
                        ]
                    else:
                        ydst = yd[
                            j * P + HD * hi : j * P + HD * (hi + 1),
                            t * SQT : (t + 1) * SQT,
                        ]
                    nc.sync.dma_start(out=ydst, in_=st[0:HD, :])

            with (
                tc.tile_pool(name="qkv_in", bufs=1) as qkv_in,
                tc.tile_pool(name="wstream", bufs=WS_BUFS) as wstream_,
            ):
                wstream = wstream_
                xTs = qkv_in.tile([P, KD, S], F32R)
                wvs = qkv_in.tile([P, KD, GCOLS], F32R)
                qk_tile.xTs = xTs

                # ---- input loads: interleave wv/xT per contraction chunk
                # so the V matmul chain starts as soon as chunk 0 lands ----
                wvr = wv.rearrange("(k p) c -> k p c", p=P)
                xTr = xT.rearrange("(k p) s -> k p s", p=P)
                for k in range(KD):
                    nc.sync.dma_start(out=wvs[:, k, :], in_=wvr[k])
                    nc.sync.dma_start(out=xTs[:, k, :], in_=xTr[k])
                onesrow = qkv_in.tile([P, NT16 * HG], F32)
                nc.vector.memset(onesrow, 1.0)
                nc.vector.tensor_copy(
                    out=V[:, :, :, HD : HD + 1],
                    in_=onesrow.rearrange(
                        "p (t h one) -> p t h one", t=NT16, one=1
                    ),
                )

                # ---- V = x @ wv  (natural [s, vcol] layout) ----
                for t in range(NT16):
                    ps = ps_sc.tile([P, GCOLS], F32, name="ps_v", tag="sc")
                    for k in range(KD):
                        nc.tensor.matmul(
                            ps,
                            lhsT=_r(xTs[:, k, t * P : (t + 1) * P]),
                            rhs=_r(wvs[:, k, :]),
                            start=(k == 0),
                            stop=(k == KD - 1),
                        )
                    nc.scalar.copy(
                        out=V[:, t, :, 0:HD],
                        in_=ps.rearrange("p (h d) -> p h d", h=HG),
                    )

                for t in range(NT4):
                    qk_tile(0, t)
                for j in range(NJ - 1):
                    for t in range(NT4):
                        attn_tile(j, t)
                        qk_tile(j + 1, t)

            # ---- last head-pair + projection, overlapped ----
            with (
                tc.tile_pool(name="late", bufs=1) as late,
                tc.tile_pool(name="projin", bufs=PJ_BUFS) as projin,
                tc.tile_pool(name="outst", bufs=PJ_BUFS) as outst,
            ):
                WP = late.tile([P, NJ, D], F32R)
                nc.sync.dma_start(out=WP, in_=wp.rearrange("(j p) d -> p j d", p=P))
                Y3 = late.tile([P, S], F32R)
                attn_tile.y3 = Y3
                ydr = yd.rearrange("(j p) s -> p j s", p=P)

                def proj_tile(t):
                    yt = projin.tile([P, NJ - 1, P], F32R, name="yt")
                    nc.sync.dma_start(
                        out=yt, in_=ydr[:, 0 : NJ - 1, t * P : (t + 1) * P]
                    )
                    for n in range(D // SQT):
                        pp = ps_sc.tile([P, SQT], F32, name="pp", tag="sc")
                        for j in range(NJ):
                            lhsT = (
                                yt[:, j, :]
                                if j < NJ - 1
                                else Y3[:, t * P : (t + 1) * P]
                            )
                            nc.tensor.matmul(
                                pp,
                                lhsT=_r(lhsT),
                                rhs=_r(WP[:, j, n * SQT : (n + 1) * SQT]),
                                start=(j == 0),
                                stop=(j == NJ - 1),
                            )
                        ot = outst.tile([P, SQT], F32, name="ot")
                        nc.scalar.copy(out=ot, in_=pp)
                        nc.sync.dma_start(
                            out=out[t * P : (t + 1) * P, n * SQT : (n + 1) * SQT],
                            in_=ot,
                        )

                for t in range(NT4):
                    attn_tile(NJ - 1, t)
                    if t > 0:
                        for tp in range(4 * (t - 1), 4 * t):
                            proj_tile(tp)
                for tp in range(4 * (NT4 - 1), 4 * NT4):
                    proj_tile(tp)
    nc.compile()
    return nc


_NC_CACHE = {}


def _get_nc(S=2048):
    if S not in _NC_CACHE:
        _NC_CACHE[S] = build_nc(S)
    return _NC_CACHE[S]


def make_masks():
    i = np.arange(P)[:, None]
    j2 = np.arange(SQT)[None, :]
    mk = np.zeros((P, 4, SQT), dtype=np.float32)
    for m in range(4):
        mk[:, m, :] = (j2 >= P * m + i).astype(np.float32)
    return mk


def shard_inputs(x, w_qkv, w_proj):
    mk = make_masks()
    ins = []
    for c in range(8):
        b, g = divmod(c, 2)
        ins.append(
            {
                "xT": np.ascontiguousarray(x[b].T),
                "wq": np.ascontiguousarray(w_qkv[:, g * GCOLS : (g + 1) * GCOLS]),
                "wk": np.ascontiguousarray(
                    w_qkv[:, D + g * GCOLS : D + (g + 1) * GCOLS]
                ),
                "wv": np.ascontiguousarray(
                    w_qkv[:, 2 * D + g * GCOLS : 2 * D + (g + 1) * GCOLS]
                ),
                "wp": np.ascontiguousarray(w_proj[g * GCOLS : (g + 1) * GCOLS, :]),
                "mk": mk,
            }
        )
    return ins


_LAST_RESULT = None


def kernel(x, w_qkv, w_proj):
    global _LAST_RESULT
    x = np.asarray(x, dtype=np.float32)
    w_qkv = np.asarray(w_qkv, dtype=np.float32)
    w_proj = np.asarray(w_proj, dtype=np.float32)
    S = x.shape[1]
    nc = _get_nc(S)
    ins = shard_inputs(x, w_qkv, w_proj)
    res = run_bass_kernel_spmd(
        nc,
        ins,
        core_ids=list(range(8)),
        trace=TRACE,
        **TRACE_KWARGS,
    )
    _LAST_RESULT = res
    outs = [res.results[c]["out"] for c in range(8)]
    return np.stack([outs[2 * b] + outs[2 * b + 1] for b in range(4)])

